# revision 21
# baseline (speedup 1.0000x reference)
"""Trainium2 Bass kernel for a dense transformer layer (attention + FFN + 2 LayerNorms).

Problem shapes: x [4, 2048, 1024], d_model=1024, heads=16 (hd=64), d_ff=4096.

Sharding: 8 cores; core c handles batch b = c//2, sequence half = c%2
(1024 query tokens).  Each core computes K/V for its batch's full 2048
tokens (duplicated across the pair — avoids any cross-core communication).
The host permutes each core's x so its own token half comes first; since
attention sums over key tokens, K/V token order is irrelevant as long as
K and V agree.

Layout strategy: activations are feature-major ("transposed": d_model on
partitions, tokens on free dim) so weight matrices serve directly as
matmul stationary operands (out = lhsT.T @ rhs).  Attention computes
S^T = K @ Q^T per head (key tokens on partitions), exp on the scalar
engine, then ctx^T = V_aug^T @ P^T where V carries a ones-column that
yields the softmax denominator for free.  All matmuls use float32r
(full-rate fp32 PE streaming, ~1e-4 relative rounding).

K^T, V (token-major) and ctx^T bounce through HBM to bound SBUF pressure.
"""

import os
import numpy as np

import concourse.bass as bass
import concourse.tile as tile
from concourse import bacc, mybir
from concourse import bass_utils

BF16 = mybir.dt.bfloat16
F32 = mybir.dt.float32
AF = mybir.ActivationFunctionType
OP = mybir.AluOpType

D = 1024          # d_model
S = 2048          # full sequence per batch
T = 1024          # query tokens per core
H = 16            # heads
HD = 64           # head dim
F = 4096          # ffn hidden
P = 128
DT = D // P       # 8 feature tiles
KT = S // P       # 16 key-token tiles
FT = F // P       # 32 hidden tiles
N_CORES = 8
EPS = 1e-5

_CACHED = {}


def _build_program():
    nc = bacc.Bacc("TRN2", target_bir_lowering=False, debug=False,
                   num_devices=N_CORES)

    tens = {}

    def di(name, shape, dtype=BF16):
        tens[name] = nc.dram_tensor(name, shape, dtype, kind="ExternalInput")

    di("xT", [D, S])
    di("wq", [D, D]); di("wk", [D, D]); di("wv", [D, D]); di("wo", [D, D])
    di("w1", [D, F]); di("w2", [F, D])
    for nm in ["bq_p", "bk_p", "bv_p", "bo_p", "b2_p", "g1_p", "be1_p"]:
        di(nm, [P, DT], F32)
    di("b1_p", [P, FT], F32)
    di("g2_d", [D], F32); di("be2_d", [D], F32)
    di("ident_d", [P, P]); di("ones_row_d", [1, P]); di("ones_col_d", [P, 1])
    di("sel_d", [H, D])
    di("ones16_d", [P, H, 1])
    tens["out"] = nc.dram_tensor("out", [T, D], F32, kind="ExternalOutput")

    with tile.TileContext(nc) as tc:
        _trace_kernel(nc, tc, tens)
    nc.compile()
    return nc


def _trace_kernel(nc, tc, t):
    xT, wq, wk, wv, wo, w1, w2 = (t["xT"], t["wq"], t["wk"], t["wv"], t["wo"],
                                  t["w1"], t["w2"])
    out = t["out"]

    from contextlib import ExitStack
    es = ExitStack()
    with es:
        dram = es.enter_context(tc.tile_pool(name="dram", bufs=1, space="DRAM"))
        kT_hbm = dram.tile([D, S], BF16, tag="kh", name="kh")
        v_hbm = dram.tile([S, H, HD + 1], BF16, tag="vh", name="vh")
        ctx_hbm = dram.tile([H * (HD + 1), T], BF16, tag="ch", name="ch")

        const = es.enter_context(tc.tile_pool(name="const", bufs=1))
        ident = const.tile([P, P], BF16, tag="ident", name="ident")
        nc.sync.dma_start(out=ident, in_=t["ident_d"][:, :])
        ones_row = const.tile([1, P], BF16, tag="onesr", name="onesr")
        nc.sync.dma_start(out=ones_row, in_=t["ones_row_d"][:, :])
        ones_col = const.tile([P, 1], BF16, tag="onesc", name="onesc")
        nc.sync.dma_start(out=ones_col, in_=t["ones_col_d"][:, :])
        ones16 = const.tile([P, H, 1], BF16, tag="ones16", name="ones16")
        nc.sync.dma_start(out=ones16, in_=t["ones16_d"][:, :, :])
        sel_sb = const.tile([H, D], BF16, tag="sel", name="sel")
        nc.sync.dma_start(out=sel_sb, in_=t["sel_d"][:, :])
        biases = {}
        for name in ["bq_p", "bk_p", "bv_p", "bo_p", "b2_p", "g1_p", "be1_p"]:
            bt = const.tile([P, DT], F32, tag=name)
            nc.sync.dma_start(out=bt, in_=t[name][:, :])
            biases[name] = bt
        eps_sb = const.tile([P, 1], F32, tag="eps", name="eps")
        nc.vector.memset(eps_sb[:], EPS)
        b1_sb = const.tile([P, FT], F32, tag="b1", name="b1")
        nc.sync.dma_start(out=b1_sb, in_=t["b1_p"][:, :])
        hT_pool = es.enter_context(tc.tile_pool(name="hT", bufs=1))
        hT = [hT_pool.tile([P, T], BF16, tag=f"hT{i}", name=f"hT{i}") for i in range(DT)]

        ctxs_pool = es.enter_context(tc.tile_pool(name="ctxs", bufs=1))
        ctxs = [ctxs_pool.tile([P, T], BF16, tag=f"ctx{i}", name=f"ctx{i}")
                for i in range(DT)]
        den16 = ctxs_pool.tile([H, T], BF16, tag="den16", name="den16")

        # =============== Phase 1: QKV projections =======================
        qt_cm = tc.tile_pool(name="qt", bufs=1)
        qt_pool = qt_cm.__enter__()
        QT = [qt_pool.tile([P, T], BF16, tag=f"qt{i}", name=f"qt{i}") for i in range(DT)]

        with tc.tile_pool(name="xsb", bufs=1) as xsb_pool, \
             tc.tile_pool(name="wsb", bufs=DT) as wsb_pool, \
             tc.tile_pool(name="p1ev", bufs=2) as ev_pool, \
             tc.tile_pool(name="vsb", bufs=4) as vsb_pool, \
             tc.tile_pool(name="psA", bufs=3, space="PSUM") as psA, \
             tc.tile_pool(name="psT", bufs=4, space="PSUM") as psT:
            xsb = []
            for dt_ in range(DT):
                xt_ = xsb_pool.tile([P, S], BF16, tag=f"x{dt_}", name=f"x{dt_}")
                nc.sync.dma_start(out=xt_, in_=xT[dt_ * P:(dt_ + 1) * P, :])
                xsb.append(xt_)

            def proj_psum(ps, w_sb, dout, cols):
                for din in range(DT):
                    nc.tensor.matmul(
                        ps[:], w_sb[din][:, dout * P:(dout + 1) * P],
                        xsb[din][:, cols], start=(din == 0),
                        stop=(din == DT - 1))

            def load_w(wd):
                w_sb = [wsb_pool.tile([P, D], BF16, tag="w", name="w") for _ in range(DT)]
                for dt_ in range(DT):
                    nc.sync.dma_start(out=w_sb[dt_],
                                      in_=wd[dt_ * P:(dt_ + 1) * P, :])
                return w_sb

            # --- V projection -> transpose -> v_hbm ---------------------
            wv_sb = load_w(wv)
            for ch in range(S // 512):           # 4 chunks of 512 tokens
                vtiles = [vsb_pool.tile([P, H, HD + 1], BF16, tag="vsb", name="vsb")
                          for _ in range(4)]
                for dout in range(DT):
                    ps = psA.tile([P, 512], F32, tag="psA", name="psA")
                    proj_psum(ps, wv_sb, dout, slice(ch * 512, (ch + 1) * 512))
                    vt = ev_pool.tile([P, 512], BF16, tag="vt", name="vt")
                    nc.scalar.activation(vt[:], ps[:], AF.Identity,
                                         bias=biases["bv_p"][:, dout:dout + 1])
                    for hh in range(2):
                        h = 2 * dout + hh
                        idsl = ident[hh * HD:(hh + 1) * HD,
                                     hh * HD:(hh + 1) * HD]
                        for st in range(4):
                            pt = psT.tile([P, HD], BF16, tag="psT", name="psT")
                            nc.tensor.transpose(
                                pt[:], vt[hh * HD:(hh + 1) * HD,
                                          st * P:(st + 1) * P],
                                idsl)
                            nc.vector.tensor_copy(vtiles[st][:, h, 0:HD],
                                                  pt[:])
                for st in range(4):
                    nc.vector.tensor_copy(vtiles[st][:, :, HD:HD + 1],
                                          ones16[:, :, :])
                    kt0 = ch * 4 + st
                    nc.sync.dma_start(
                        out=v_hbm[kt0 * P:(kt0 + 1) * P, :, :],
                        in_=vtiles[st][:, :, :])

            # --- K projection -> kT_hbm ---------------------------------
            wk_sb = load_w(wk)
            for ch in range(S // 512):
                for dout in range(DT):
                    ps = psA.tile([P, 512], F32, tag="psA", name="psA")
                    proj_psum(ps, wk_sb, dout, slice(ch * 512, (ch + 1) * 512))
                    kt_ = ev_pool.tile([P, 512], BF16, tag="kt", name="kt")
                    nc.scalar.activation(kt_[:], ps[:], AF.Identity,
                                         bias=biases["bk_p"][:, dout:dout + 1])
                    nc.sync.dma_start(
                        out=kT_hbm[dout * P:(dout + 1) * P,
                                   ch * 512:(ch + 1) * 512],
                        in_=kt_[:])

            # --- Q projection (own tokens = xT cols 0:1024; resident) ---
            wq_sb = load_w(wq)
            for ch in range(T // 512):           # 2 chunks
                for dout in range(DT):
                    ps = psA.tile([P, 512], F32, tag="psA", name="psA")
                    proj_psum(ps, wq_sb, dout, slice(ch * 512, (ch + 1) * 512))
                    nc.scalar.activation(QT[dout][:, ch * 512:(ch + 1) * 512],
                                         ps[:], AF.Identity,
                                         bias=biases["bq_p"][:, dout:dout + 1])

        # =============== Phase 2: attention =============================
        with tc.tile_pool(name="kbuf", bufs=2) as kbuf_pool, \
             tc.tile_pool(name="vbuf", bufs=2 * KT) as vbuf_pool, \
             tc.tile_pool(name="pbuf", bufs=4) as pbuf_pool, \
             tc.tile_pool(name="cev", bufs=3) as cev_pool, \
             tc.tile_pool(name="psS", bufs=2, space="PSUM") as psS, \
             tc.tile_pool(name="psC", bufs=4, space="PSUM") as psC:
            for dt_ in range(DT):        # head pair: heads 2dt_, 2dt_+1
                ksb = kbuf_pool.tile([P, S], BF16, tag="kb", name="kb")
                nc.sync.dma_start(out=ksb,
                                  in_=kT_hbm[dt_ * P:(dt_ + 1) * P, :])
                vsb = [vbuf_pool.tile([P, 2, HD + 1], BF16, tag="vb",
                                      name="vb") for _ in range(KT)]
                for j in range(KT):
                    nc.sync.dma_start(
                        out=vsb[j],
                        in_=v_hbm[j * P:(j + 1) * P, 2 * dt_:2 * dt_ + 2, :])
                for hh in range(2):
                    h = 2 * dt_ + hh
                    r0 = hh * HD
                    self_attention_head(nc, QT, dt_, r0, hh, ksb, vsb,
                                        cev_pool, pbuf_pool, psS, psC,
                                        ctx_hbm, h)
                # prefetch this pair's ctx + denom rows back from HBM
                for hh in range(2):
                    h = 2 * dt_ + hh
                    nc.sync.dma_start(
                        out=ctxs[dt_][hh * HD:(hh + 1) * HD, :],
                        in_=ctx_hbm[h * (HD + 1):h * (HD + 1) + HD, :])
                nc.sync.dma_start(
                    out=den16[2 * dt_:2 * dt_ + 2, :],
                    in_=bass.AP(
                        tensor=ctx_hbm.tensor,
                        offset=ctx_hbm.offset + (2 * dt_ * (HD + 1) + HD) * T,
                        ap=[[(HD + 1) * T, 2], [1, T]]))


def self_attention_head(nc, QT, dt_, r0, hh, ksb, vsb, cev_pool, pbuf_pool,
                        psS, psC, ctx_hbm, h):
    KT_, P_, HD_, T_ = KT, P, HD, T
    NQC = T_ // 512
    # stage the head's Q rows for both query chunks; sibling rows zero so
    # the sibling K rows in the full [128,128] stationary tiles cancel
    qstgs, cpss = [], []
    for qc in range(NQC):
        qstg = cev_pool.tile([P_, 512], BF16, tag="qstg", name="qstg")
        nc.vector.memset(qstg[:], 0.0)
        nc.vector.tensor_copy(
            qstg[r0:r0 + HD_, :],
            QT[dt_][r0:r0 + HD_, qc * 512:(qc + 1) * 512])
        qstgs.append(qstg)
        cpss.append(psC.tile([HD_ + 1, 512], F32, tag="cps", name="cps"))
    # interleave the two independent qc streams to keep PE and ACT full
    for jp in range(KT_ // 2):
        j0, j1 = 2 * jp, 2 * jp + 1
        for qc in range(NQC):
            sps = psS.tile([P_, 2, 512], F32, tag="sps", name="sps")
            nc.tensor.matmul(sps[:, 0, :], ksb[:, j0 * P_:(j0 + 1) * P_],
                             qstgs[qc][:], start=True, stop=True)
            nc.tensor.matmul(sps[:, 1, :], ksb[:, j1 * P_:(j1 + 1) * P_],
                             qstgs[qc][:], start=True, stop=True)
            pT = pbuf_pool.tile([P_, 2, 512], BF16, tag="pT", name="pT")
            nc.scalar.activation(pT[:], sps[:], AF.Exp)
            nc.tensor.matmul(cpss[qc][:], vsb[j0][:, hh, :], pT[:, 0, :],
                             start=(jp == 0), stop=False)
            nc.tensor.matmul(cpss[qc][:], vsb[j1][:, hh, :], pT[:, 1, :],
                             start=False, stop=(jp == KT_ // 2 - 1))
    for qc in range(NQC):
        # evict unnormalized ctx + denom row; normalization happens in
        # one batched pass at the start of the Wo phase
        ctx_sb = cev_pool.tile([HD_ + 1, 512], BF16, tag="ctxe", name="ctxe")
        nc.vector.tensor_copy(ctx_sb[:], cpss[qc][:])
        nc.sync.dma_start(
            out=ctx_hbm[h * (HD_ + 1):(h + 1) * (HD_ + 1),
                        qc * 512:(qc + 1) * 512],
            in_=ctx_sb[:])

        # =============== Phase 3: Wo + residual + LN1 ===================
        wx_pool = es.enter_context(tc.tile_pool(name="wx", bufs=40))
        with tc.tile_pool(name="xq", bufs=1) as xq_pool, \
             tc.tile_pool(name="zT", bufs=1) as zT_pool, \
             tc.tile_pool(name="ln1", bufs=1) as ln1_pool, \
             tc.tile_pool(name="psW", bufs=3, space="PSUM") as psW, \
             tc.tile_pool(name="psStat", bufs=1, space="PSUM") as psStat, \
             tc.tile_pool(name="psBc", bufs=1, space="PSUM") as psBc:
            rcp16f = ctxs_pool.tile([H, T], F32, tag="rcp16f", name="rcp16f")
            nc.vector.reciprocal(rcp16f[:], den16[:])
            rcp16 = ctxs_pool.tile([H, T], BF16, tag="rcp16", name="rcp16")
            nc.vector.tensor_copy(rcp16[:], rcp16f[:])
            # broadcast each head pair's reciprocal rows to 128 partitions
            # via a constant selector matmul, then scale ctx in place
            for dt_ in range(DT):
                for ch in range(T // 512):
                    bcp = psBc.tile([P, 512], F32, tag="bcp", name="bcp")
                    nc.tensor.matmul(
                        bcp[:], sel_sb[:, dt_ * P:(dt_ + 1) * P],
                        rcp16[:, ch * 512:(ch + 1) * 512],
                        start=True, stop=True)
                    nc.vector.tensor_tensor(
                        out=ctxs[dt_][:, ch * 512:(ch + 1) * 512],
                        in0=ctxs[dt_][:, ch * 512:(ch + 1) * 512],
                        in1=bcp[:], op=OP.mult)
            xq = [xq_pool.tile([P, T], BF16, tag=f"xq{i}", name=f"xq{i}") for i in range(DT)]
            for dt_ in range(DT):
                nc.sync.dma_start(
                    out=xq[dt_],
                    in_=xT[dt_ * P:(dt_ + 1) * P, 0:T])
            wo_sb = [wx_pool.tile([P, D], BF16, tag="wx", name="wx") for _ in range(DT)]
            for dt_ in range(DT):
                nc.sync.dma_start(out=wo_sb[dt_],
                                  in_=wo[dt_ * P:(dt_ + 1) * P, :])
            zT = [zT_pool.tile([P, T], BF16, tag=f"zT{i}", name=f"zT{i}") for i in range(DT)]
            for ch in range(T // 512):
                for dout in range(DT):
                    ps = psW.tile([P, 512], F32, tag="psW", name="psW")
                    for din in range(DT):
                        nc.tensor.matmul(
                            ps[:], wo_sb[din][:, dout * P:(dout + 1) * P],
                            ctxs[din][:, ch * 512:(ch + 1) * 512],
                            start=(din == 0), stop=(din == DT - 1))
                    # z = attn_out + bo + x_resid
                    nc.vector.scalar_tensor_tensor(
                        zT[dout][:, ch * 512:(ch + 1) * 512], ps[:],
                        biases["bo_p"][:, dout:dout + 1],
                        xq[dout][:, ch * 512:(ch + 1) * 512],
                        op0=OP.add, op1=OP.add)

            # ---- LN1 (feature-major; stats over partitions via PE) -----
            for ch in range(T // 512):
                sl = slice(ch * 512, (ch + 1) * 512)
                sum_ps = psStat.tile([1, 512], F32, tag="s", name="s")
                sq_ps = psStat.tile([1, 512], F32, tag="q", name="q")
                for dt_ in range(DT):
                    zsq = ln1_pool.tile([P, 512], BF16, tag="zsq", name="zsq")
                    nc.vector.tensor_tensor(
                        out=zsq[:], in0=zT[dt_][:, sl],
                        in1=zT[dt_][:, sl], op=OP.mult)
                    nc.tensor.matmul(sum_ps[:], ones_col[:], zT[dt_][:, sl],
                                     start=(dt_ == 0), stop=(dt_ == DT - 1))
                    nc.tensor.matmul(sq_ps[:], ones_col[:], zsq[:],
                                     start=(dt_ == 0), stop=(dt_ == DT - 1))
                mean = ln1_pool.tile([1, 512], F32, tag="mean", name="mean")
                nc.scalar.mul(mean[:], sum_ps[:], 1.0 / D)
                msq = ln1_pool.tile([1, 512], F32, tag="msq", name="msq")
                nc.scalar.mul(msq[:], sq_ps[:], 1.0 / D)
                m2 = ln1_pool.tile([1, 512], F32, tag="m2", name="m2")
                nc.vector.tensor_mul(m2[:], mean[:], mean[:])
                var = ln1_pool.tile([1, 512], F32, tag="var", name="var")
                nc.vector.tensor_sub(var[:], msq[:], m2[:])
                std = ln1_pool.tile([1, 512], F32, tag="std", name="std")
                nc.scalar.activation(std[:], var[:], AF.Sqrt, bias=eps_sb[0:1, :])
                rstd = ln1_pool.tile([1, 512], F32, tag="rstd", name="rstd")
                nc.vector.reciprocal(rstd[:], std[:])
                mean_r = ln1_pool.tile([1, 512], BF16, tag="meanr", name="meanr")
                nc.vector.tensor_copy(mean_r[:], mean[:])
                rstd_r = ln1_pool.tile([1, 512], BF16, tag="rstdr", name="rstdr")
                nc.vector.tensor_copy(rstd_r[:], rstd[:])
                bm_ps = psBc.tile([P, 512], F32, tag="bm", name="bm")
                nc.tensor.matmul(bm_ps[:], ones_row[:], mean_r[:],
                                 start=True, stop=True)
                br_ps = psBc.tile([P, 512], F32, tag="br", name="br")
                nc.tensor.matmul(br_ps[:], ones_row[:], rstd_r[:],
                                 start=True, stop=True)
                bm = ln1_pool.tile([P, 512], F32, tag="bm_sb", name="bm_sb")
                nc.scalar.copy(bm[:], bm_ps[:])
                br = ln1_pool.tile([P, 512], F32, tag="br_sb", name="br_sb")
                nc.scalar.copy(br[:], br_ps[:])
                for dt_ in range(DT):
                    tmp = ln1_pool.tile([P, 512], F32, tag="n1", name="n1")
                    nc.vector.scalar_tensor_tensor(
                        tmp[:], zT[dt_][:, sl],
                        1.0, bm[:], op0=OP.mult, op1=OP.subtract)
                    tmp2 = ln1_pool.tile([P, 512], F32, tag="n2", name="n2")
                    nc.vector.scalar_tensor_tensor(
                        tmp2[:], tmp[:],
                        biases["g1_p"][:, dt_:dt_ + 1], br[:],
                        op0=OP.mult, op1=OP.mult)
                    nc.vector.tensor_scalar(
                        out=hT[dt_][:, sl], in0=tmp2[:],
                        scalar1=biases["be1_p"][:, dt_:dt_ + 1], scalar2=None,
                        op0=OP.add)

        # =============== Phase 4: FFN + residual ========================
        with tc.tile_pool(name="z2T", bufs=1) as z2T_pool:
            z2T = [z2T_pool.tile([P, T], BF16, tag=f"z2T{i}", name=f"z2T{i}")
                   for i in range(DT)]
            with tc.tile_pool(name="t1", bufs=12) as t1_pool, \
                 tc.tile_pool(name="o2", bufs=1) as o2_pool, \
                 tc.tile_pool(name="psF1", bufs=3, space="PSUM") as psF1, \
                 tc.tile_pool(name="psF2", bufs=3, space="PSUM") as psF2:
                out2 = [o2_pool.tile([P, T], F32, tag=f"o2{i}", name=f"o2{i}")
                        for i in range(DT)]
                for hb in range(4):              # hidden blocks of 1024
                    w1b = [wx_pool.tile([P, D], BF16, tag="wx", name="wx")
                           for _ in range(DT)]
                    for i in range(DT):
                        nc.sync.dma_start(
                            out=w1b[i],
                            in_=w1[i * P:(i + 1) * P,
                                   hb * 1024:(hb + 1) * 1024])
                    w2b = [wx_pool.tile([P, D], BF16, tag="wx", name="wx")
                           for _ in range(DT)]
                    for i in range(DT):
                        nc.sync.dma_start(
                            out=w2b[i],
                            in_=w2[(hb * 8 + i) * P:(hb * 8 + i + 1) * P, :])
                    for tc4 in range(T // 512):  # 2 token chunks of 512
                        tsl = slice(tc4 * 512, (tc4 + 1) * 512)
                        t1s = []
                        for i in range(DT):      # 8 hidden tiles in block
                            t1ps = psF1.tile([P, 512], F32, tag="t1ps", name="t1ps")
                            for din in range(DT):
                                nc.tensor.matmul(
                                    t1ps[:], w1b[din][:, i * P:(i + 1) * P],
                                    hT[din][:, tsl],
                                    start=(din == 0), stop=(din == DT - 1))
                            t1 = t1_pool.tile([P, 512], BF16, tag="t1", name="t1")
                            nc.scalar.activation(
                                t1[:], t1ps[:], AF.Relu,
                                bias=b1_sb[:, hb * 8 + i:hb * 8 + i + 1])
                            t1s.append(t1)
                        for dout in range(DT):
                            o2ps = psF2.tile([P, 512], F32, tag="o2ps", name="o2ps")
                            for i in range(DT):
                                nc.tensor.matmul(
                                    o2ps[:], w2b[i][:, dout * P:(dout + 1) * P],
                                    t1s[i][:],
                                    start=(i == 0), stop=(i == DT - 1))
                            if hb == 0:
                                nc.vector.tensor_copy(out2[dout][:, tsl],
                                                      o2ps[:])
                            else:
                                nc.vector.tensor_tensor(
                                    out=out2[dout][:, tsl], in0=o2ps[:],
                                    in1=out2[dout][:, tsl], op=OP.add)
                # z2 = ffn_out + b2 + h   (residual)
                for dt_ in range(DT):
                    nc.vector.scalar_tensor_tensor(
                        z2T[dt_][:], out2[dt_][:],
                        biases["b2_p"][:, dt_:dt_ + 1],
                        hT[dt_][:], op0=OP.add, op1=OP.add)

            # =============== Phase 5: transpose + LN2 + out =============
            with tc.tile_pool(name="tm", bufs=2) as tm_pool, \
                 tc.tile_pool(name="ln2", bufs=2) as ln2_pool, \
                 tc.tile_pool(name="psT5", bufs=4, space="PSUM") as psT5:
                g2_bc = ln2_pool.tile([P, D], F32, tag="g2bc", name="g2bc")
                nc.sync.dma_start(out=g2_bc, in_=bass.AP(
                    tensor=t["g2_d"], offset=0, ap=[[0, P], [1, D]]))
                be2_bc = ln2_pool.tile([P, D], F32, tag="be2bc", name="be2bc")
                nc.sync.dma_start(out=be2_bc, in_=bass.AP(
                    tensor=t["be2_d"], offset=0, ap=[[0, P], [1, D]]))
                for nt in range(DT):             # 8 token tiles of 128
                    z2 = tm_pool.tile([P, D], F32, tag="z2tm", name="z2tm")
                    for dt_ in range(DT):
                        pt = psT5.tile([P, P], BF16, tag="psT5", name="psT5")
                        nc.tensor.transpose(
                            pt[:], z2T[dt_][:, nt * P:(nt + 1) * P], ident[:])
                        nc.scalar.copy(z2[:, dt_ * P:(dt_ + 1) * P],
                                       pt[:])
                    stats = ln2_pool.tile([P, 2, 6], F32, tag="st", name="st")
                    for g in range(2):
                        nc.vector.bn_stats(out=stats[:, g, :],
                                           in_=z2[:, g * 512:(g + 1) * 512])
                    mv = ln2_pool.tile([P, 2], F32, tag="mv", name="mv")
                    nc.vector.bn_aggr(out=mv[:], in_=stats[:])
                    std = ln2_pool.tile([P, 1], F32, tag="std2", name="std2")
                    nc.scalar.activation(std[:], mv[:, 1:2], AF.Sqrt, bias=eps_sb[:])
                    rstd = ln2_pool.tile([P, 1], F32, tag="rstd2", name="rstd2")
                    nc.vector.reciprocal(rstd[:], std[:])
                    xn = ln2_pool.tile([P, D], F32, tag="xn", name="xn")
                    nc.vector.tensor_scalar(
                        out=xn[:], in0=z2[:], scalar1=mv[:, 0:1],
                        scalar2=rstd[:], op0=OP.subtract, op1=OP.mult)
                    xg = ln2_pool.tile([P, D], F32, tag="xg", name="xg")
                    nc.vector.tensor_mul(xg[:], xn[:], g2_bc[:])
                    fin = ln2_pool.tile([P, D], F32, tag="fin", name="fin")
                    nc.vector.tensor_add(fin[:], xg[:], be2_bc[:])
                    nc.sync.dma_start(out=out[nt * P:(nt + 1) * P, :],
                                      in_=fin[:])


def _selector():
    sel = np.zeros((H, D), dtype=np.float32)
    for dt_ in range(DT):
        for m in range(P):
            sel[2 * dt_ + m // HD, dt_ * P + m] = 1.0
    return sel


def _pack(v, nt):
    return np.ascontiguousarray(v.reshape(nt, P).T)


def kernel(x, Wq, bq, Wk, bk, Wv, bv, Wo, bo, W1, b1, W2, b2, g1, beta1,
           g2, beta2):
    x = np.asarray(x, dtype=np.float32)
    if "nc" not in _CACHED:
        _CACHED["nc"] = _build_program()
    nc = _CACHED["nc"]

    import ml_dtypes
    bf16 = lambda a: np.ascontiguousarray(
        np.asarray(a, dtype=np.float32).astype(ml_dtypes.bfloat16))
    f32 = lambda a: np.ascontiguousarray(np.asarray(a, dtype=np.float32))
    scale = 1.0 / np.sqrt(HD)
    common = {
        "wq": bf16(Wq), "wk": bf16(np.asarray(Wk, np.float64) * scale), "wv": bf16(Wv),
        "wo": bf16(Wo), "w1": bf16(W1), "w2": bf16(W2),
        "bq_p": _pack(f32(bq), DT), "bk_p": _pack(f32(bk) * scale, DT),
        "bv_p": _pack(f32(bv), DT), "bo_p": _pack(f32(bo), DT),
        "b1_p": _pack(f32(b1), FT), "b2_p": _pack(f32(b2), DT),
        "g1_p": _pack(f32(g1), DT), "be1_p": _pack(f32(beta1), DT),
        "g2_d": f32(g2), "be2_d": f32(beta2),
        "ident_d": np.eye(P).astype(ml_dtypes.bfloat16),
        "ones_row_d": np.ones((1, P)).astype(ml_dtypes.bfloat16),
        "ones_col_d": np.ones((P, 1)).astype(ml_dtypes.bfloat16),
        "ones16_d": np.ones((P, H, 1)).astype(ml_dtypes.bfloat16),
        "sel_d": _selector().astype(ml_dtypes.bfloat16),
    }
    in_maps = []
    for c in range(N_CORES):
        b, half = c // 2, c % 2
        own = x[b, half * T:(half + 1) * T]           # [1024, 1024]
        other = x[b, (1 - half) * T:(2 - half) * T]
        xT_c = np.ascontiguousarray(
            np.concatenate([own, other], axis=0).T).astype(
                ml_dtypes.bfloat16)                   # [1024, 2048]
        in_maps.append({**common, "xT": xT_c})

    trace = bool(os.environ.get("KERNEL_TRACE"))
    res = bass_utils.run_bass_kernel_spmd(
        nc, in_maps, core_ids=list(range(N_CORES)), trace=trace)
    _CACHED["last_result"] = res

    y = np.empty((4, S, D), dtype=np.float32)
    for c in range(N_CORES):
        b, half = c // 2, c % 2
        y[b, half * T:(half + 1) * T] = res.results[c]["out"]
    return y
